# revision 6
# baseline (speedup 1.0000x reference)
"""TRN2 Bass kernel for nn_Brain: delayed-synapse recurrent network.

Strategy (dense delay-batched "futures"):
  total_input[t] = c0 + sum_{d=1}^{15} W_d @ acts_{t-d}   (acts_s, s>=1)
  acts_t = tanh(total_input[t])
- Edges with delay >= 16 never fire (valid = delay < t <= 16): dropped.
- delay-0 edges always read hist[0] (python history[-0] quirk) => per-neuron
  constant c0, computed on host from the input row.
- W_d stored dense [4096 src, 512 tgt] bf16 per core (8-way target shard);
  both batch rows ride the same weight stream as extra matmul columns.
- Bucket d may batch up to d consecutive steps in one application (the
  contribution to step t uses acts_{t-d}, available d-1 steps early), so
  bucket d streams ceil((16-d)/d) times instead of 16-d. d<=3 stay
  SBUF-resident; d>=4 stream from HBM double-buffered.
- PSUM accumulates per-step totals; per step: +c0, tanh (ScalarE),
  AllGather the 512-target slice across the 8 cores, land it in an SBUF
  activation-history tile that feeds later matmuls via affine APs.
"""
import numpy as np

N_NEURONS = 4096
INPUT_SIZE = 1024
BATCH = 2
STEPS = 16
N_CORES = 8
TGT_PER_CORE = N_NEURONS // N_CORES        # 512
TCH = TGT_PER_CORE // 128                  # 4 target chunks per core
SCH = N_NEURONS // 128                     # 32 source chunks
MAXD = STEPS - 1                           # delays 1..15 useful
RESIDENT_D = (1, 2, 3)

_compiled = None


def _schedule():
    """Apps: (d, s0, nb) -> contributes to steps t in [s0+d, s0+d+nb-1]
    using acts_{s0..s0+nb-1}. Ready after acts_{s0+nb-1}."""
    apps = []
    for d in range(1, MAXD + 1):
        nsteps = STEPS - d           # t = d+1..16 -> s = 1..16-d
        b = d                        # max batch = d
        s0 = 1
        while s0 <= nsteps:
            nb = min(b, nsteps - s0 + 1)
            apps.append((d, s0, nb))
            s0 += nb
    return apps


def _build_program():
    from concourse import bacc, mybir, tile

    dt = mybir.dt
    nc = bacc.Bacc(None, target_bir_lowering=False, debug=False)

    # ---- dram params (identical program on all cores; data differs) ----
    wd_in = {}
    for d in range(1, MAXD + 1):
        wd_in[d] = nc.declare_dram_parameter(
            f"wd{d}", [128, SCH * TCH * 128], dt.bfloat16, isOutput=False)
    c0_in = nc.declare_dram_parameter("c0", [128, TCH * BATCH], dt.float32,
                                      isOutput=False)
    out_d = nc.declare_dram_parameter("out", [128, TCH * BATCH], dt.float32,
                                      isOutput=True)

    # collective bounce buffers (internal DRAM; shared out for allgather)
    cc_in = nc.dram_tensor("cc_in", [128, TCH * BATCH], dt.bfloat16)
    cc_out = nc.dram_tensor("cc_out", [N_CORES * 128, TCH * BATCH],
                            dt.bfloat16, addr_space="Shared")

    apps = _schedule()
    # group apps by ready time: ready[s] = apps ready right after acts_s
    ready = {s: [] for s in range(0, STEPS + 1)}
    for (d, s0, nb) in apps:
        ready[s0 + nb - 1].append((d, s0, nb))
    # all apps with s0 == 1, nb == given... note ready-time >= 1.

    HCOLS = MAXD * SCH * BATCH  # acts_hist free cols: (s-1, c, r)

    with tile.TileContext(nc) as tc:
        with (
            tc.tile_pool(name="wres", bufs=1) as wres_pool,
            tc.tile_pool(name="wstream", bufs=2) as wstream_pool,
            tc.tile_pool(name="aux", bufs=1) as aux_pool,
            tc.tile_pool(name="psum", bufs=1, space="PSUM") as psum_pool,
        ):
            # resident weights d=1..3: [128, SCH*TCH*128] bf16 (4MB each)
            t_wres = {}
            for d in RESIDENT_D:
                t_wres[d] = wres_pool.tile([128, SCH * TCH * 128], dt.bfloat16,
                                           name=f"wres{d}", tag=f"wres{d}")
            t_c0 = aux_pool.tile([128, TCH * BATCH], dt.float32)
            t_hist = aux_pool.tile([128, HCOLS], dt.bfloat16)
            t_pre = aux_pool.tile([128, TCH * BATCH], dt.float32)
            t_act = aux_pool.tile([128, TCH * BATCH], dt.float32)
            t_actb = aux_pool.tile([128, TCH * BATCH], dt.bfloat16)
            t_ps = [psum_pool.tile([128, STEPS * BATCH], dt.float32,
                                   name=f"ps{tc_i}", tag=f"ps{tc_i}")
                    for tc_i in range(TCH)]

            # loads
            nc.sync.dma_start(t_c0[:], c0_in[:])
            for d in RESIDENT_D:
                nc.sync.dma_start(t_wres[d][:], wd_in[d][:])

            # preload c0 into every step column of psum (stride-0 expand)
            for tc_i in range(TCH):
                c0_slice = t_c0[:, tc_i * BATCH:(tc_i + 1) * BATCH]
                c0_exp = c0_slice.rearrange("p (o r) -> p o r", o=1)\
                    .broadcast_to([128, STEPS, BATCH])
                nc.vector.tensor_copy(
                    t_ps[tc_i][:].rearrange("p (t r) -> p t r", r=BATCH),
                    c0_exp)

            def run_app(d, s0, nb):
                # weight tile source
                if d in RESIDENT_D:
                    t_w = t_wres[d]
                else:
                    t_w = wstream_pool.tile([128, SCH * TCH * 128],
                                            dt.bfloat16, name="wstream",
                                            tag="wstream")
                    nc.sync.dma_start(t_w[:], wd_in[d][:])
                w3 = t_w[:].rearrange("p (sc tcch m) -> p sc tcch m",
                                      sc=SCH, tcch=TCH)
                t0 = s0 + d  # first target step
                for tc_i in range(TCH):
                    # psum cols for steps t0..t0+nb-1, both rows
                    ps_ap = t_ps[tc_i][:].rearrange(
                        "p (t r) -> p t r", r=BATCH)[:, t0 - 1:t0 - 1 + nb, :]
                    for sc in range(SCH):
                        rhs = t_hist[:].rearrange(
                            "p (s c r) -> p s c r", s=MAXD, c=SCH
                        )[:, s0 - 1:s0 - 1 + nb, sc, :]
                        nc.tensor.matmul(
                            ps_ap, w3[:, sc, tc_i, :], rhs,
                            start=False, stop=False, skip_group_check=True)

            for t in range(1, STEPS + 1):
                # epilogue for step t: all apps contributing to t have been
                # issued in earlier iterations (ready <= t-1); Tile's dep
                # tracking orders psum reads after those matmuls.
                sc_ctx = nc.named_scope(f"step{t:02d}")
                sc_ctx.__enter__()
                for tc_i in range(TCH):
                    nc.scalar.activation(
                        t_actb[:, tc_i * BATCH:(tc_i + 1) * BATCH],
                        t_ps[tc_i][:].rearrange("p (t r) -> p t r", r=BATCH)
                        [:, t - 1, :],
                        mybir.ActivationFunctionType.Tanh)
                if t == STEPS:
                    nc.vector.tensor_copy(t_act[:], t_actb[:])
                    nc.sync.dma_start(out_d[:], t_act[:])
                    sc_ctx.__exit__(None, None, None)
                    break
                # allgather acts_t slices across 8 cores
                nc.sync.dma_start(cc_in[:], t_actb[:])
                nc.gpsimd.collective_compute(
                    "AllGather", mybir.AluOpType.bypass,
                    replica_groups=[list(range(N_CORES))],
                    ins=[cc_in[:]], outs=[cc_out[:]])
                # land into history: hist[p, (s=t, c=4j+tc, r)]
                src_ap = cc_out[:].rearrange(
                    "(j p) (tcch r) -> p j tcch r", p=128, r=BATCH)
                dst_ap = t_hist[:].rearrange(
                    "p (s c r) -> p s c r", s=MAXD, c=SCH
                )[:, t - 1, :, :].rearrange(
                    "p (j tcch) r -> p j tcch r", j=N_CORES)
                nc.sync.dma_start(dst_ap, src_ap)
                sc_ctx.__exit__(None, None, None)
                # issue apps that became ready with acts_t
                for (d, s0, nb) in ready.get(t, []):
                    with nc.named_scope(f"app_d{d}_s{s0}"):
                        run_app(d, s0, nb)

    nc.compile()
    return nc


def _preprocess(input_data, connection_weights, connection_indices,
                delay_values, steps):
    """Host: build per-core dense bucketed weights, c0, initial acts."""
    assert steps == STEPS
    w = np.asarray(connection_weights, np.float32)
    ci = np.asarray(connection_indices)
    dl = np.asarray(delay_values)
    src, tgt = ci[0].astype(np.int64), ci[1].astype(np.int64)
    x = np.asarray(input_data, np.float32)           # [BATCH, 1024]

    acts0 = np.zeros((BATCH, N_NEURONS), np.float32)
    acts0[:, :INPUT_SIZE] = x

    # c0: delay-0 edges always read acts0[src]
    m0 = dl == 0
    c0 = np.zeros((BATCH, N_NEURONS), np.float32)
    for r in range(BATCH):
        np.add.at(c0[r], tgt[m0], w[m0] * acts0[r, src[m0]])

    # dense W_d [4096 src, 4096 tgt] fp32 -> bf16, per bucket
    import ml_dtypes
    wds = {}
    for d in range(1, MAXD + 1):
        md = dl == d
        Wd = np.zeros((N_NEURONS, N_NEURONS), np.float32)
        np.add.at(Wd, (src[md], tgt[md]), w[md])
        wds[d] = Wd.astype(ml_dtypes.bfloat16)

    in_maps = []
    for k in range(N_CORES):
        t0, t1 = k * TGT_PER_CORE, (k + 1) * TGT_PER_CORE
        im = {}
        for d in range(1, MAXD + 1):
            Ws = wds[d][:, t0:t1]                      # [4096, 512]
            # [(sc p), (tc m)] -> [p, (sc, tc, m)]
            Wr = Ws.reshape(SCH, 128, TCH, 128).transpose(1, 0, 2, 3)
            im[f"wd{d}"] = np.ascontiguousarray(
                Wr.reshape(128, SCH * TCH * 128))
        # c0 arranged [p, (tc, r)]
        c0k = np.zeros((128, TCH * BATCH), np.float32)
        for tci in range(TCH):
            for r in range(BATCH):
                c0k[:, tci * BATCH + r] = c0[r, t0 + tci * 128:
                                             t0 + (tci + 1) * 128]
        im["c0"] = c0k
        in_maps.append(im)
    return in_maps


def kernel(input_data, connection_weights, connection_indices,
           delay_values, steps):
    global _compiled
    from concourse.bass_utils import run_bass_kernel_spmd

    in_maps = _preprocess(input_data, connection_weights,
                          connection_indices, delay_values, int(steps))
    if _compiled is None:
        _compiled = _build_program()
    res = run_bass_kernel_spmd(_compiled, in_maps, list(range(N_CORES)))

    out = np.zeros((BATCH, N_NEURONS), np.float32)
    for k in range(N_CORES):
        o = res.results[k]["out"]                      # [128, (tc, r)]
        t0 = k * TGT_PER_CORE
        for tci in range(TCH):
            for r in range(BATCH):
                out[r, t0 + tci * 128: t0 + (tci + 1) * 128] = \
                    o[:, tci * BATCH + r]
    return out[:, -INPUT_SIZE:].astype(np.float32)


# revision 7
# speedup vs baseline: 1.1037x; 1.1037x over previous
"""TRN2 Bass kernel for nn_Brain: delayed-synapse recurrent network.

Strategy (dense delay-batched "futures"):
  total_input[t] = c0 + sum_{d=1}^{15} W_d @ acts_{t-d}   (acts_s, s>=1)
  acts_t = tanh(total_input[t])
- Edges with delay >= 16 never fire (valid = delay < t <= 16): dropped.
- delay-0 edges always read hist[0] (python history[-0] quirk) => per-neuron
  constant c0, computed on host from the input row.
- W_d stored dense [4096 src, 512 tgt] bf16 per core (8-way target shard);
  both batch rows ride the same weight stream as extra matmul columns.
- Bucket d may batch up to d consecutive steps in one application (the
  contribution to step t uses acts_{t-d}, available d-1 steps early), so
  bucket d streams ceil((16-d)/d) times instead of 16-d. d<=3 stay
  SBUF-resident; d>=4 stream from HBM double-buffered.
- PSUM accumulates per-step totals; per step: +c0, tanh (ScalarE),
  AllGather the 512-target slice across the 8 cores, land it in an SBUF
  activation-history tile that feeds later matmuls via affine APs.
"""
import numpy as np

N_NEURONS = 4096
INPUT_SIZE = 1024
BATCH = 2
STEPS = 16
N_CORES = 8
TGT_PER_CORE = N_NEURONS // N_CORES        # 512
TCH = TGT_PER_CORE // 128                  # 4 target chunks per core
SCH = N_NEURONS // 128                     # 32 source chunks
MAXD = STEPS - 1                           # delays 1..15 useful
RESIDENT_D = (1, 2, 3)

_compiled = None


def _schedule():
    """Apps: (d, s0, nb) -> contributes to steps t in [s0+d, s0+d+nb-1]
    using acts_{s0..s0+nb-1}. Ready after acts_{s0+nb-1}."""
    apps = []
    for d in range(1, MAXD + 1):
        nsteps = STEPS - d           # t = d+1..16 -> s = 1..16-d
        b = d                        # max batch = d
        s0 = 1
        while s0 <= nsteps:
            nb = min(b, nsteps - s0 + 1)
            apps.append((d, s0, nb))
            s0 += nb
    return apps


def _build_program():
    from concourse import bacc, mybir, tile

    dt = mybir.dt
    nc = bacc.Bacc(None, target_bir_lowering=False, debug=False)

    # ---- dram params (identical program on all cores; data differs) ----
    wd_in = {}
    for d in range(1, MAXD + 1):
        wd_in[d] = nc.declare_dram_parameter(
            f"wd{d}", [128, SCH * TCH * 128], dt.bfloat16, isOutput=False)
    c0m_in = nc.declare_dram_parameter("c0mat", [128, TCH * 128], dt.float32,
                                       isOutput=False)
    sel_in = nc.declare_dram_parameter("sel", [128, STEPS * BATCH], dt.float32,
                                       isOutput=False)
    out_d = nc.declare_dram_parameter("out", [128, TCH * BATCH], dt.float32,
                                      isOutput=True)

    # collective bounce buffers (internal DRAM; shared out for allgather)
    cc_in = nc.dram_tensor("cc_in", [128, TCH * BATCH], dt.bfloat16)
    cc_out = nc.dram_tensor("cc_out", [N_CORES * 128, TCH * BATCH],
                            dt.bfloat16, addr_space="Shared")

    apps = _schedule()
    # group apps by ready time: ready[s] = apps ready right after acts_s
    ready = {s: [] for s in range(0, STEPS + 1)}
    for (d, s0, nb) in apps:
        ready[s0 + nb - 1].append((d, s0, nb))
    # all apps with s0 == 1, nb == given... note ready-time >= 1.

    HCOLS = MAXD * SCH * BATCH  # acts_hist free cols: (s-1, c, r)

    with tile.TileContext(nc) as tc:
        with (
            tc.tile_pool(name="wres", bufs=1) as wres_pool,
            tc.tile_pool(name="wstream", bufs=2) as wstream_pool,
            tc.tile_pool(name="aux", bufs=1) as aux_pool,
            tc.tile_pool(name="psum", bufs=1, space="PSUM") as psum_pool,
        ):
            # resident weights d=1..3: [128, SCH*TCH*128] bf16 (4MB each)
            t_wres = {}
            for d in RESIDENT_D:
                t_wres[d] = wres_pool.tile([128, SCH * TCH * 128], dt.bfloat16,
                                           name=f"wres{d}", tag=f"wres{d}")
            t_c0m = aux_pool.tile([128, TCH * 128], dt.float32)
            t_sel = aux_pool.tile([128, STEPS * BATCH], dt.float32)
            t_hist = aux_pool.tile([128, HCOLS], dt.bfloat16)
            t_pre = aux_pool.tile([128, TCH * BATCH], dt.float32)
            t_act = aux_pool.tile([128, TCH * BATCH], dt.float32)
            t_actb = aux_pool.tile([128, TCH * BATCH], dt.bfloat16)
            t_ps = [psum_pool.tile([128, STEPS * BATCH], dt.float32,
                                   name=f"ps{tc_i}", tag=f"ps{tc_i}")
                    for tc_i in range(TCH)]

            # loads
            nc.sync.dma_start(t_c0m[:], c0m_in[:])
            nc.sync.dma_start(t_sel[:], sel_in[:])
            for d in RESIDENT_D:
                nc.sync.dma_start(t_wres[d][:], wd_in[d][:])

            # initialize psum with c0 in every step column via a
            # selector matmul: out[m,(t,r)] = c0mat[r, m] (rows >= BATCH zero)
            for tc_i in range(TCH):
                nc.tensor.matmul(
                    t_ps[tc_i][:], t_c0m[:, tc_i * 128:(tc_i + 1) * 128],
                    t_sel[:], start=True, stop=False, skip_group_check=True)

            def run_app(d, s0, nb):
                # weight tile source
                if d in RESIDENT_D:
                    t_w = t_wres[d]
                else:
                    t_w = wstream_pool.tile([128, SCH * TCH * 128],
                                            dt.bfloat16, name="wstream",
                                            tag="wstream")
                    nc.sync.dma_start(t_w[:], wd_in[d][:])
                w3 = t_w[:].rearrange("p (sc tcch m) -> p sc tcch m",
                                      sc=SCH, tcch=TCH)
                t0 = s0 + d  # first target step
                for tc_i in range(TCH):
                    # psum cols for steps t0..t0+nb-1, both rows
                    ps_ap = t_ps[tc_i][:].rearrange(
                        "p (t r) -> p t r", r=BATCH)[:, t0 - 1:t0 - 1 + nb, :]
                    for sc in range(SCH):
                        rhs = t_hist[:].rearrange(
                            "p (s c r) -> p s c r", s=MAXD, c=SCH
                        )[:, s0 - 1:s0 - 1 + nb, sc, :]
                        nc.tensor.matmul(
                            ps_ap, w3[:, sc, tc_i, :], rhs,
                            start=False, stop=False, skip_group_check=True)

            for t in range(1, STEPS + 1):
                # epilogue for step t: all apps contributing to t have been
                # issued in earlier iterations (ready <= t-1); Tile's dep
                # tracking orders psum reads after those matmuls.
                sc_ctx = nc.named_scope(f"step{t:02d}")
                sc_ctx.__enter__()
                for tc_i in range(TCH):
                    nc.scalar.activation(
                        t_actb[:, tc_i * BATCH:(tc_i + 1) * BATCH],
                        t_ps[tc_i][:].rearrange("p (t r) -> p t r", r=BATCH)
                        [:, t - 1, :],
                        mybir.ActivationFunctionType.Tanh)
                if t == STEPS:
                    for tc_i in range(TCH):
                        nc.scalar.activation(
                            t_act[:, tc_i * BATCH:(tc_i + 1) * BATCH],
                            t_ps[tc_i][:].rearrange("p (t r) -> p t r",
                                                    r=BATCH)[:, t - 1, :],
                            mybir.ActivationFunctionType.Tanh)
                    nc.sync.dma_start(out_d[:], t_act[:])
                    sc_ctx.__exit__(None, None, None)
                    break
                # allgather acts_t slices across 8 cores
                nc.sync.dma_start(cc_in[:], t_actb[:])
                nc.gpsimd.collective_compute(
                    "AllGather", mybir.AluOpType.bypass,
                    replica_groups=[list(range(N_CORES))],
                    ins=[cc_in[:]], outs=[cc_out[:]])
                # land into history: hist[p, (s=t, c=4j+tc, r)]
                src_ap = cc_out[:].rearrange(
                    "(j p) (tcch r) -> p j tcch r", p=128, r=BATCH)
                dst_ap = t_hist[:].rearrange(
                    "p (s c r) -> p s c r", s=MAXD, c=SCH
                )[:, t - 1, :, :].rearrange(
                    "p (j tcch) r -> p j tcch r", j=N_CORES)
                nc.sync.dma_start(dst_ap, src_ap)
                sc_ctx.__exit__(None, None, None)
                # issue apps that became ready with acts_t
                for (d, s0, nb) in ready.get(t, []):
                    with nc.named_scope(f"app_d{d}_s{s0}"):
                        run_app(d, s0, nb)

    nc.compile()
    return nc


def _preprocess(input_data, connection_weights, connection_indices,
                delay_values, steps):
    """Host: build per-core dense bucketed weights, c0, initial acts."""
    assert steps == STEPS
    w = np.asarray(connection_weights, np.float32)
    ci = np.asarray(connection_indices)
    dl = np.asarray(delay_values)
    src, tgt = ci[0].astype(np.int64), ci[1].astype(np.int64)
    x = np.asarray(input_data, np.float32)           # [BATCH, 1024]

    acts0 = np.zeros((BATCH, N_NEURONS), np.float32)
    acts0[:, :INPUT_SIZE] = x

    # c0: delay-0 edges always read acts0[src]
    m0 = dl == 0
    c0 = np.zeros((BATCH, N_NEURONS), np.float32)
    for r in range(BATCH):
        np.add.at(c0[r], tgt[m0], w[m0] * acts0[r, src[m0]])

    # dense W_d [4096 src, 4096 tgt] fp32 -> bf16, per bucket
    import ml_dtypes
    wds = {}
    for d in range(1, MAXD + 1):
        md = dl == d
        Wd = np.zeros((N_NEURONS, N_NEURONS), np.float32)
        np.add.at(Wd, (src[md], tgt[md]), w[md])
        wds[d] = Wd.astype(ml_dtypes.bfloat16)

    in_maps = []
    for k in range(N_CORES):
        t0, t1 = k * TGT_PER_CORE, (k + 1) * TGT_PER_CORE
        im = {}
        for d in range(1, MAXD + 1):
            Ws = wds[d][:, t0:t1]                      # [4096, 512]
            # [(sc p), (tc m)] -> [p, (sc, tc, m)]
            Wr = Ws.reshape(SCH, 128, TCH, 128).transpose(1, 0, 2, 3)
            im[f"wd{d}"] = np.ascontiguousarray(
                Wr.reshape(128, SCH * TCH * 128))
        # c0mat[p, (tc, m)] = c0[p-th row, target] for p < BATCH else 0
        c0m = np.zeros((128, TCH * 128), np.float32)
        for r in range(BATCH):
            c0m[r] = c0[r, t0:t1].reshape(TCH * 128)
        im["c0mat"] = c0m
        sel = np.zeros((128, STEPS * BATCH), np.float32)
        for r in range(BATCH):
            sel[r, r::BATCH] = 1.0
        im["sel"] = sel
        in_maps.append(im)
    return in_maps


def kernel(input_data, connection_weights, connection_indices,
           delay_values, steps):
    global _compiled
    from concourse.bass_utils import run_bass_kernel_spmd

    in_maps = _preprocess(input_data, connection_weights,
                          connection_indices, delay_values, int(steps))
    if _compiled is None:
        _compiled = _build_program()
    res = run_bass_kernel_spmd(_compiled, in_maps, list(range(N_CORES)))

    out = np.zeros((BATCH, N_NEURONS), np.float32)
    for k in range(N_CORES):
        o = res.results[k]["out"]                      # [128, (tc, r)]
        t0 = k * TGT_PER_CORE
        for tci in range(TCH):
            for r in range(BATCH):
                out[r, t0 + tci * 128: t0 + (tci + 1) * 128] = \
                    o[:, tci * BATCH + r]
    return out[:, -INPUT_SIZE:].astype(np.float32)


# revision 8
# speedup vs baseline: 1.1631x; 1.0538x over previous
"""TRN2 Bass kernel for nn_Brain: delayed-synapse recurrent network.

Strategy (dense delay-batched "futures"):
  total_input[t] = c0 + sum_{d=1}^{15} W_d @ acts_{t-d}   (acts_s, s>=1)
  acts_t = tanh(total_input[t])
- Edges with delay >= 16 never fire (valid = delay < t <= 16): dropped.
- delay-0 edges always read hist[0] (python history[-0] quirk) => per-neuron
  constant c0, computed on host from the input row.
- W_d stored dense [4096 src, 512 tgt] bf16 per core (8-way target shard);
  both batch rows ride the same weight stream as extra matmul columns.
- Bucket d may batch up to d consecutive steps in one application (the
  contribution to step t uses acts_{t-d}, available d-1 steps early), so
  bucket d streams ceil((16-d)/d) times instead of 16-d. d<=3 stay
  SBUF-resident; d>=4 stream from HBM double-buffered.
- PSUM accumulates per-step totals; per step: +c0, tanh (ScalarE),
  AllGather the 512-target slice across the 8 cores, land it in an SBUF
  activation-history tile that feeds later matmuls via affine APs.
"""
import numpy as np

N_NEURONS = 4096
INPUT_SIZE = 1024
BATCH = 2
STEPS = 16
N_CORES = 8
TGT_PER_CORE = N_NEURONS // N_CORES        # 512
TCH = TGT_PER_CORE // 128                  # 4 target chunks per core
SCH = N_NEURONS // 128                     # 32 source chunks
MAXD = STEPS - 1                           # delays 1..15 useful
RESIDENT_D = (1, 2, 3)

_compiled = None


def _schedule():
    """Apps: (d, s0, nb) -> contributes to steps t in [s0+d, s0+d+nb-1]
    using acts_{s0..s0+nb-1}. Ready after acts_{s0+nb-1}."""
    apps = []
    for d in range(1, MAXD + 1):
        nsteps = STEPS - d           # t = d+1..16 -> s = 1..16-d
        b = d                        # max batch = d
        s0 = 1
        while s0 <= nsteps:
            nb = min(b, nsteps - s0 + 1)
            apps.append((d, s0, nb))
            s0 += nb
    return apps


def _build_program():
    from concourse import bacc, mybir, tile

    dt = mybir.dt
    nc = bacc.Bacc(None, target_bir_lowering=False, debug=False)

    # ---- dram params (identical program on all cores; data differs) ----
    wd_in = {}
    for d in range(1, MAXD + 1):
        wd_in[d] = nc.declare_dram_parameter(
            f"wd{d}", [128, SCH * TCH * 128], dt.bfloat16, isOutput=False)
    c0m_in = nc.declare_dram_parameter("c0mat", [128, TCH * 128], dt.float32,
                                       isOutput=False)
    sel_in = nc.declare_dram_parameter("sel", [128, STEPS * BATCH], dt.float32,
                                       isOutput=False)
    out_d = nc.declare_dram_parameter("out", [128, TCH * BATCH], dt.float32,
                                      isOutput=True)

    # collective bounce buffers (internal DRAM; shared out for allgather)
    cc_in = nc.dram_tensor("cc_in", [128, TCH * BATCH], dt.bfloat16)
    cc_out = nc.dram_tensor("cc_out", [N_CORES * 128, TCH * BATCH],
                            dt.bfloat16, addr_space="Shared")

    apps = _schedule()
    # group apps by ready time: ready[s] = apps ready right after acts_s
    ready = {s: [] for s in range(0, STEPS + 1)}
    for (d, s0, nb) in apps:
        ready[s0 + nb - 1].append((d, s0, nb))
    # all apps with s0 == 1, nb == given... note ready-time >= 1.

    HCOLS = MAXD * SCH * BATCH  # acts_hist free cols: (s-1, c, r)

    with tile.TileContext(nc) as tc:
        with (
            tc.tile_pool(name="wres", bufs=1) as wres_pool,
            tc.tile_pool(name="wstream", bufs=3) as wstream_pool,
            tc.tile_pool(name="aux", bufs=1) as aux_pool,
            tc.tile_pool(name="psum", bufs=1, space="PSUM") as psum_pool,
        ):
            # resident weights d=1..3: [128, SCH*TCH*128] bf16 (4MB each)
            t_wres = {}
            for d in RESIDENT_D:
                t_wres[d] = wres_pool.tile([128, SCH * TCH * 128], dt.bfloat16,
                                           name=f"wres{d}", tag=f"wres{d}")
            t_c0m = aux_pool.tile([128, TCH * 128], dt.float32)
            t_sel = aux_pool.tile([128, STEPS * BATCH], dt.float32)
            t_hist = aux_pool.tile([128, HCOLS], dt.bfloat16)
            t_pre = aux_pool.tile([128, TCH * BATCH], dt.float32)
            t_act = aux_pool.tile([128, TCH * BATCH], dt.float32)
            t_actb = aux_pool.tile([128, TCH * BATCH], dt.bfloat16)
            t_psall = psum_pool.tile([128, TCH * 512], dt.float32,
                                     name="psall", tag="psall")
            ps4 = t_psall[:].rearrange("p (tcch b) -> p tcch b", tcch=TCH)

            # loads
            nc.sync.dma_start(t_c0m[:], c0m_in[:])
            nc.sync.dma_start(t_sel[:], sel_in[:])
            for d in RESIDENT_D:
                nc.sync.dma_start(t_wres[d][:], wd_in[d][:])

            # initialize psum with c0 in every step column via a
            # selector matmul: out[m,(t,r)] = c0mat[r, m] (rows >= BATCH zero)
            for tc_i in range(TCH):
                nc.tensor.matmul(
                    ps4[:, tc_i, :STEPS * BATCH],
                    t_c0m[:, tc_i * 128:(tc_i + 1) * 128],
                    t_sel[:], start=True, stop=False, skip_group_check=True)

            def run_app(d, s0, nb):
                # weight tile source
                if d in RESIDENT_D:
                    t_w = t_wres[d]
                else:
                    t_w = wstream_pool.tile([128, SCH * TCH * 128],
                                            dt.bfloat16, name="wstream",
                                            tag="wstream")
                    nc.sync.dma_start(t_w[:], wd_in[d][:])
                w3 = t_w[:].rearrange("p (sc tcch m) -> p sc tcch m",
                                      sc=SCH, tcch=TCH)
                t0 = s0 + d  # first target step
                for tc_i in range(TCH):
                    # psum cols for steps t0..t0+nb-1, both rows
                    ps_ap = ps4[:, tc_i, :STEPS * BATCH].rearrange(
                        "p (t r) -> p t r", r=BATCH)[:, t0 - 1:t0 - 1 + nb, :]
                    for sc in range(SCH):
                        rhs = t_hist[:].rearrange(
                            "p (s c r) -> p s c r", s=MAXD, c=SCH
                        )[:, s0 - 1:s0 - 1 + nb, sc, :]
                        nc.tensor.matmul(
                            ps_ap, w3[:, sc, tc_i, :], rhs,
                            start=False, stop=False, skip_group_check=True)

            for t in range(1, STEPS + 1):
                # epilogue for step t: all apps contributing to t have been
                # issued in earlier iterations (ready <= t-1); Tile's dep
                # tracking orders psum reads after those matmuls.
                sc_ctx = nc.named_scope(f"step{t:02d}")
                sc_ctx.__enter__()
                ps_t = t_psall[:].rearrange(
                    "p (tcch t r) -> p tcch t r", tcch=TCH, t=512 // BATCH
                )[:, :, t - 1, :]
                nc.scalar.activation(
                    t_actb[:].rearrange("p (tcch r) -> p tcch r", tcch=TCH),
                    ps_t, mybir.ActivationFunctionType.Tanh)
                if t == STEPS:
                    nc.scalar.activation(
                        t_act[:].rearrange("p (tcch r) -> p tcch r", tcch=TCH),
                        ps_t, mybir.ActivationFunctionType.Tanh)
                    nc.sync.dma_start(out_d[:], t_act[:])
                    sc_ctx.__exit__(None, None, None)
                    break
                # allgather acts_t slices across 8 cores
                nc.sync.dma_start(cc_in[:], t_actb[:])
                nc.gpsimd.collective_compute(
                    "AllGather", mybir.AluOpType.bypass,
                    replica_groups=[list(range(N_CORES))],
                    ins=[cc_in[:]], outs=[cc_out[:]])
                # land into history: hist[p, (s=t, c=4j+tc, r)]
                src_ap = cc_out[:].rearrange(
                    "(j p) (tcch r) -> p j tcch r", p=128, r=BATCH)
                dst_ap = t_hist[:].rearrange(
                    "p (s c r) -> p s c r", s=MAXD, c=SCH
                )[:, t - 1, :, :].rearrange(
                    "p (j tcch) r -> p j tcch r", j=N_CORES)
                nc.sync.dma_start(dst_ap, src_ap)
                sc_ctx.__exit__(None, None, None)
                # issue apps that became ready with acts_t
                for (d, s0, nb) in ready.get(t, []):
                    with nc.named_scope(f"app_d{d}_s{s0}"):
                        run_app(d, s0, nb)

    nc.compile()
    return nc


def _preprocess(input_data, connection_weights, connection_indices,
                delay_values, steps):
    """Host: build per-core dense bucketed weights, c0, initial acts."""
    assert steps == STEPS
    w = np.asarray(connection_weights, np.float32)
    ci = np.asarray(connection_indices)
    dl = np.asarray(delay_values)
    src, tgt = ci[0].astype(np.int64), ci[1].astype(np.int64)
    x = np.asarray(input_data, np.float32)           # [BATCH, 1024]

    acts0 = np.zeros((BATCH, N_NEURONS), np.float32)
    acts0[:, :INPUT_SIZE] = x

    # c0: delay-0 edges always read acts0[src]
    m0 = dl == 0
    c0 = np.zeros((BATCH, N_NEURONS), np.float32)
    for r in range(BATCH):
        np.add.at(c0[r], tgt[m0], w[m0] * acts0[r, src[m0]])

    # dense W_d [4096 src, 4096 tgt] fp32 -> bf16, per bucket
    import ml_dtypes
    wds = {}
    for d in range(1, MAXD + 1):
        md = dl == d
        Wd = np.zeros((N_NEURONS, N_NEURONS), np.float32)
        np.add.at(Wd, (src[md], tgt[md]), w[md])
        wds[d] = Wd.astype(ml_dtypes.bfloat16)

    in_maps = []
    for k in range(N_CORES):
        t0, t1 = k * TGT_PER_CORE, (k + 1) * TGT_PER_CORE
        im = {}
        for d in range(1, MAXD + 1):
            Ws = wds[d][:, t0:t1]                      # [4096, 512]
            # [(sc p), (tc m)] -> [p, (sc, tc, m)]
            Wr = Ws.reshape(SCH, 128, TCH, 128).transpose(1, 0, 2, 3)
            im[f"wd{d}"] = np.ascontiguousarray(
                Wr.reshape(128, SCH * TCH * 128))
        # c0mat[p, (tc, m)] = c0[p-th row, target] for p < BATCH else 0
        c0m = np.zeros((128, TCH * 128), np.float32)
        for r in range(BATCH):
            c0m[r] = c0[r, t0:t1].reshape(TCH * 128)
        im["c0mat"] = c0m
        sel = np.zeros((128, STEPS * BATCH), np.float32)
        for r in range(BATCH):
            sel[r, r::BATCH] = 1.0
        im["sel"] = sel
        in_maps.append(im)
    return in_maps


def kernel(input_data, connection_weights, connection_indices,
           delay_values, steps):
    global _compiled
    from concourse.bass_utils import run_bass_kernel_spmd

    in_maps = _preprocess(input_data, connection_weights,
                          connection_indices, delay_values, int(steps))
    if _compiled is None:
        _compiled = _build_program()
    res = run_bass_kernel_spmd(_compiled, in_maps, list(range(N_CORES)))

    out = np.zeros((BATCH, N_NEURONS), np.float32)
    for k in range(N_CORES):
        o = res.results[k]["out"]                      # [128, (tc, r)]
        t0 = k * TGT_PER_CORE
        for tci in range(TCH):
            for r in range(BATCH):
                out[r, t0 + tci * 128: t0 + (tci + 1) * 128] = \
                    o[:, tci * BATCH + r]
    return out[:, -INPUT_SIZE:].astype(np.float32)
